# revision 32
# baseline (speedup 1.0000x reference)
"""Expert-choice MoE with complex-valued experts + ModReLU, on 8 trn2 NeuronCores.

Strategy (token-sharded, no collectives):
  - Host computes the gating scores and jax.lax.top_k exactly as the reference
    does (CPU backend: reproducible, and it doubles as the routing / sharding
    decision, yielding the topk_idx / topk_scores / counts outputs directly).
  - Tokens are assigned to cores by a balancer so every (expert, core) pair
    gets exactly 128 of the expert's 1024 selected tokens -> zero GEMM padding.
  - The complex GEMM runs in split precision: x and W are each decomposed on
    the host into hi+lo bf16 parts (x = xh + xl to ~2^-18 relative); the
    device computes xh*wh + xh*wl + xl*wh with fp32 PSUM accumulation, which
    carries ~1e-5 relative error (fp32-envelope) at 3/4 the PE cost of native
    fp32 matmul, and makes x 2-byte so dma_gather(transpose=True) can gather
    AND transpose the selected tokens in one DMA (host pre-de-interleaves
    re/im so gathered partition chunks are the GEMM contraction chunks).
  - Per expert: gather-transpose xh/xl, 48 bf16 matmuls into yr/yi PSUM,
    gate-scale on the PSUM->SBUF ACT copy, then a CCE dma_scatter_add into a
    pre-zeroed per-core accumulator in HBM (expert-serialized: duplicate
    destination rows never race).
  - bias == 0 makes ModReLU the identity to within 1.5e-8 absolute
    (ratio = relu(m)/max(m,1e-8) = m/m = 1.0 exactly for m >= 1e-8, and
    |res| <= |out| < 1.5e-8 otherwise), so acc IS the result.  A general
    on-device ModReLU phase is kept for bias != 0.
  - Host scatters the 8 per-core result shards back through the balancer
    permutation.
"""

import numpy as np

B_T, D, E = 16384, 512, 16
NCORES = 8
TL = B_T // NCORES      # tokens per core
DRI = 2 * D             # re/im row length
KSEL = B_T // E         # top-k per expert
KPC = KSEL // NCORES    # per-(expert, core) target load = 128

_NC_CACHE = {}


def _round_up(v, m):
    return -(-v // m) * m


def _build(caps, bias_zero, repeat=1):
    """Build the per-core Bass program (same NEFF runs on all 8 cores).
    repeat>1 re-runs the whole accumulate pipeline (acc ends up repeat x the
    result) — used for steady-state wall-clock timing."""
    import concourse.bacc as bacc
    import concourse.mybir as mybir
    import concourse.tile as tile

    f32 = mybir.dt.float32
    bf16 = mybir.dt.bfloat16
    i16 = mybir.dt.int16
    AF = mybir.ActivationFunctionType

    ch = [-(-c // 128) for c in caps]                  # m-chunks per expert
    off16 = np.concatenate([[0], np.cumsum([c // 16 for c in caps])]).astype(int)
    choff = np.concatenate([[0], np.cumsum(ch)]).astype(int)
    GW16 = int(off16[-1])
    NCH = int(choff[-1])
    MAXCH = max(ch)
    CAPMAX = max(caps)

    nc = bacc.Bacc(None, target_bir_lowering=False)
    # x de-interleaved ([re0..511, im0..511] rows), hi/lo bf16 split packed
    # side by side: row = [xh(1024) | xl(1024)]
    x2_d = nc.dram_tensor("x2", [TL, 2 * DRI], bf16, kind="ExternalInput")
    # weights [E, 128, 16, 512] partition-major: 4 k-chunks each of
    # wrh | wrl | wih | wil, one contiguous 16KB DMA row per partition
    wa_d = nc.dram_tensor("wa", [E, 128, 16, D], bf16, kind="ExternalInput")
    gidx_d = nc.dram_tensor("gidx", [128, GW16], i16, kind="ExternalInput")
    sidx_d = nc.dram_tensor("sidx", [128, GW16], i16, kind="ExternalInput")
    sc_d = nc.dram_tensor("sc", [128, NCH], f32, kind="ExternalInput")
    bias_d = nc.dram_tensor("bias_b", [128, D], f32, kind="ExternalInput")
    # +16 trash rows: padded scatter instances land on row TL so their
    # (zero-valued) adds can never race a real token-0 add within a call
    acc_d = nc.dram_tensor("acc", [TL + 16, DRI], f32, kind="ExternalOutput")
    res_d = None
    if not bias_zero:
        res_d = nc.dram_tensor("res", [TL, DRI], f32, kind="ExternalOutput")

    with tile.TileContext(nc) as tc:
        with (
            tc.tile_pool(name="const", bufs=1) as constp,
            tc.tile_pool(name="w", bufs=2) as wp,
            tc.tile_pool(name="xt", bufs=2) as xtp,
            tc.tile_pool(name="y", bufs=3) as yp,
            tc.tile_pool(name="p2", bufs=2) as p2p,
            tc.tile_pool(name="yps", bufs=4, space="PSUM") as ypp,
        ):
            gidx_sb = constp.tile([128, GW16], i16)
            sidx_sb = constp.tile([128, GW16], i16)
            sc_sb = constp.tile([128, NCH], f32)
            nc.sync.dma_start(gidx_sb[:], gidx_d[:])
            nc.sync.dma_start(sidx_sb[:], sidx_d[:])
            nc.sync.dma_start(sc_sb[:], sc_d[:])
            bias_sb = None
            if not bias_zero:
                bias_sb = constp.tile([128, D], f32)
                nc.sync.dma_start(bias_sb[:], bias_d[:])

            for rep, e in [(r, e) for r in range(repeat) for e in range(E)]:
                cap = caps[e]
                che = ch[e]
                wa_t = wp.tile([128, 16, D], bf16, tag="wa", name=f"wa_{rep}_{e}")
                nc.sync.dma_start(wa_t[:], wa_d[e])
                w = {"wrh": wa_t[:, 0:4, :], "wrl": wa_t[:, 4:8, :],
                     "wih": wa_t[:, 8:12, :], "wil": wa_t[:, 12:16, :]}
                # negated wi parts (for yr -= xi*wi); sign-flip on DVE
                nwi_t = wp.tile([128, 8, D], bf16, tag="nwi", name=f"nwi_{rep}_{e}")
                nc.vector.tensor_scalar_mul(nwi_t[:], wa_t[:, 8:16, :], -1.0)
                w["nwih"] = nwi_t[:, 0:4, :]
                w["nwil"] = nwi_t[:, 4:8, :]

                # gather+transpose: out[p, c, i] = x2[row_i][c*128 + p]
                # c 0..3 -> xh re chunks, 4..7 -> xh im, 8..15 -> xl re/im
                x_t = xtp.tile([128, 16, cap], bf16, tag="xt", name=f"xt_{rep}_{e}")
                idx = gidx_sb[:, off16[e]:off16[e + 1]]
                nc.gpsimd.dma_gather(x_t[:], x2_d[:], idx,
                                     num_idxs=cap, num_idxs_reg=cap,
                                     elem_size=2 * DRI, transpose=True)
                xh_t = x_t[:, 0:8, :]
                xl_t = x_t[:, 8:16, :]

                y_sb = yp.tile([128, MAXCH, DRI], f32, tag="y")
                yv = y_sb[:].rearrange("p c (d two) -> p c two d", two=2)
                for j in range(che):
                    m = min(128, cap - 128 * j)
                    sl = slice(128 * j, 128 * j + m)
                    yr_ps = ypp.tile([128, D], f32, tag="y")
                    yi_ps = ypp.tile([128, D], f32, tag="y")
                    # yr = xr*wr - xi*wi, split into hi/lo cross terms
                    yr_terms = [("xh", 0, "wrh"), ("xh", 0, "wrl"),
                                ("xl", 0, "wrh"), ("xh", 4, "nwih"),
                                ("xh", 4, "nwil"), ("xl", 4, "nwih")]
                    yi_terms = [("xh", 0, "wih"), ("xh", 0, "wil"),
                                ("xl", 0, "wih"), ("xh", 4, "wrh"),
                                ("xh", 4, "wrl"), ("xl", 4, "wrh")]
                    for ps, terms in ((yr_ps, yr_terms), (yi_ps, yi_terms)):
                        n = len(terms) * 4
                        k = 0
                        for xs, coff, wn in terms:
                            xt = xh_t if xs == "xh" else xl_t
                            for kb in range(4):
                                nc.tensor.matmul(
                                    ps[:m, :], xt[:, coff + kb, sl],
                                    w[wn][:, kb, :],
                                    start=(k == 0), stop=(k == n - 1))
                                k += 1
                    col = int(choff[e]) + j
                    nc.scalar.activation(yv[:m, j, 0, :], yr_ps[:m, :], AF.Copy,
                                         scale=sc_sb[:m, col:col + 1])
                    nc.scalar.activation(yv[:m, j, 1, :], yi_ps[:m, :], AF.Copy,
                                         scale=sc_sb[:m, col:col + 1])
                mlast = cap - 128 * (che - 1)
                if mlast < 128:
                    nc.vector.memset(y_sb[:, che - 1, :], 0.0)
                # Padded instances (only when caps exceed the balanced 128)
                # carry zero y rows and scatter into the trash row TL.
                nc.gpsimd.dma_scatter_add(
                    acc_d[:],
                    y_sb[:, :che, :],
                    sidx_sb[:, off16[e]:off16[e + 1]],
                    num_idxs=cap,
                    num_idxs_reg=cap,
                    elem_size=DRI,
                )

            if not bias_zero:
                for j in range(TL // 128):
                    a_sb = p2p.tile([128, DRI], f32, tag="a")
                    nc.sync.dma_start(a_sb[:], acc_d[j * 128:(j + 1) * 128, :])
                    av = a_sb[:].rearrange("p (d two) -> p two d", two=2)
                    re, im = av[:, 0, :], av[:, 1, :]
                    m2r = p2p.tile([128, D], f32, tag="m2r")
                    m2i = p2p.tile([128, D], f32, tag="m2i")
                    nc.scalar.square(m2r[:], re)
                    nc.scalar.square(m2i[:], im)
                    m2 = p2p.tile([128, D], f32, tag="m2")
                    nc.vector.tensor_add(m2[:], m2r[:], m2i[:])
                    mag = p2p.tile([128, D], f32, tag="mag")
                    nc.scalar.sqrt(mag[:], m2[:])
                    ratio = p2p.tile([128, D], f32, tag="ratio")
                    t = p2p.tile([128, D], f32, tag="t")
                    nc.vector.tensor_add(t[:], mag[:], bias_sb[:])
                    nc.vector.tensor_scalar_max(t[:], t[:], 0.0)
                    sm = p2p.tile([128, D], f32, tag="sm")
                    nc.vector.tensor_scalar_max(sm[:], mag[:], 1e-8)
                    rcp = p2p.tile([128, D], f32, tag="rcp")
                    nc.vector.reciprocal(rcp[:], sm[:])
                    nc.vector.tensor_mul(ratio[:], t[:], rcp[:])
                    res_sb = p2p.tile([128, DRI], f32, tag="res")
                    rv = res_sb[:].rearrange("p (d two) -> p two d", two=2)
                    nc.vector.tensor_mul(rv[:, 0, :], re, ratio[:])
                    nc.vector.tensor_mul(rv[:, 1, :], im, ratio[:])
                    nc.sync.dma_start(res_d[j * 128:(j + 1) * 128, :], res_sb[:])

    nc.finalize()
    return nc


def _host_routing(x, gate_weights):
    """Mirror the reference gating (same jax ops). Pinned to the CPU backend:
    the neuron eager matmul is not run-to-run deterministic, so CPU gives a
    reproducible routing that exactly matches a CPU-run reference."""
    import jax
    import jax.numpy as jnp

    b_t, d, _ = x.shape
    e = gate_weights.shape[1]
    k = max(1, b_t // e)
    try:
        cpu = jax.devices("cpu")[0]
    except Exception:
        cpu = None
    ctx = jax.default_device(cpu) if cpu is not None else _nullcontext()
    with ctx:
        scores = jnp.asarray(x).reshape(b_t, 2 * d) @ jnp.asarray(gate_weights)
        topk_scores, topk_idx = jax.lax.top_k(scores.T, k)
        return np.asarray(topk_scores), np.asarray(topk_idx)


class _nullcontext:
    def __enter__(self):
        return None

    def __exit__(self, *a):
        return False


def _balance(ti):
    """Assign tokens to cores (2048 each) so each expert's 1024 selected
    tokens split as close to 128-per-core as possible (eliminates GEMM
    padding). Returns perm[NCORES][TL] = global token ids per core slot."""
    members = [[] for _ in range(B_T)]
    for ei in range(E):
        for t in ti[ei]:
            members[int(t)].append(ei)
    active = [t for t in range(B_T) if members[t]]
    active.sort(key=lambda t: -len(members[t]))
    load = np.zeros((NCORES, E), np.int64)
    ntok = np.zeros(NCORES, np.int64)
    assign = np.full(B_T, -1, np.int64)
    for t in active:
        es = members[t]
        best, bkey = None, None
        for c in range(NCORES):
            if ntok[c] >= TL:
                continue
            if all(load[c][e] < KPC for e in es):
                key = (max(load[c][e] for e in es), ntok[c])
                if best is None or key < bkey:
                    best, bkey = c, key
        if best is None:
            # overflow fallback: least-overloaded core (caps grow past 128)
            best = min((c for c in range(NCORES) if ntok[c] < TL),
                       key=lambda c: sum(max(0, load[c][e] + 1 - KPC)
                                         for e in es))
        assign[t] = best
        ntok[best] += 1
        for e in es:
            load[best][e] += 1
    free = iter([t for t in range(B_T) if assign[t] < 0])
    perm = []
    for c in range(NCORES):
        mine = [t for t in range(B_T) if assign[t] == c]
        while len(mine) < TL:
            mine.append(next(free))
        perm.append(np.sort(np.array(mine, np.int64)))
    return perm


def _split_bf16(a):
    import ml_dtypes
    hi = a.astype(ml_dtypes.bfloat16)
    lo = (a - hi.astype(np.float32)).astype(ml_dtypes.bfloat16)
    return hi, lo


def prepare(x, gate_weights, experts_weight, bias):
    """Host-side routing + input packing. Returns (caps, bias_zero, in_maps,
    perm, tv, ti, counts)."""
    x = np.ascontiguousarray(np.asarray(x, dtype=np.float32))
    gate_weights = np.asarray(gate_weights, dtype=np.float32)
    experts_weight = np.asarray(experts_weight, dtype=np.float32)
    bias = np.asarray(bias, dtype=np.float32)

    tv, ti = _host_routing(x, gate_weights)
    counts = np.bincount(ti.reshape(-1), minlength=B_T).astype(np.float32)
    w_inst = (tv / np.maximum(counts[ti], 1.0)).astype(np.float32)

    perm = _balance(ti)
    tok_core = np.empty(B_T, np.int64)
    tok_slot = np.empty(B_T, np.int64)
    for c in range(NCORES):
        tok_core[perm[c]] = c
        tok_slot[perm[c]] = np.arange(TL)

    core = tok_core[ti]
    loc = tok_slot[ti]

    sel = [[None] * E for _ in range(NCORES)]
    selw = [[None] * E for _ in range(NCORES)]
    cnt_ec = np.zeros((NCORES, E), np.int64)
    for ei in range(E):
        for c in range(NCORES):
            msk = core[ei] == c
            l = loc[ei][msk]
            w = w_inst[ei][msk]
            o = np.argsort(l, kind="stable")
            sel[c][ei] = l[o].astype(np.int64)
            selw[c][ei] = w[o]
            cnt_ec[c, ei] = l.size

    # gather(transpose=True) requires num_idxs % 128 == 0
    caps = tuple(int(max(128, _round_up(int(cnt_ec[:, ei].max()), 128)))
                 for ei in range(E))
    ch = [-(-c // 128) for c in caps]
    off16 = np.concatenate([[0], np.cumsum([c // 16 for c in caps])]).astype(int)
    choff = np.concatenate([[0], np.cumsum(ch)]).astype(int)
    GW16 = int(off16[-1])
    NCH = int(choff[-1])

    # weights: [E, D, D] -> [E, 128, 4, D] partition-major, then hi/lo bf16,
    # packed as [E, 128, 16, D] = per-partition [wrh | wrl | wih | wil]
    wr = np.ascontiguousarray(
        experts_weight[:, :, :, 0].reshape(E, 4, 128, D).transpose(0, 2, 1, 3))
    wi = np.ascontiguousarray(
        experts_weight[:, :, :, 1].reshape(E, 4, 128, D).transpose(0, 2, 1, 3))
    wrh, wrl = _split_bf16(wr)
    wih, wil = _split_bf16(wi)
    wa = np.ascontiguousarray(
        np.concatenate([wrh, wrl, wih, wil], axis=2))

    # x: de-interleave re/im per row, hi/lo bf16 split packed side by side
    x_de = np.ascontiguousarray(
        x.reshape(B_T, D, 2).transpose(0, 2, 1).reshape(B_T, DRI))
    xh_all, xl_all = _split_bf16(x_de)
    x2_all = np.ascontiguousarray(np.concatenate([xh_all, xl_all], axis=1))

    bias_zero = bool(np.all(bias == 0.0))
    bias_b = np.broadcast_to(bias[None, :], (128, D)).copy()

    in_maps = []
    for c in range(NCORES):
        gidx = np.zeros((16, GW16), np.int16)
        sidx = np.full((16, GW16), TL, np.int16)  # pads -> trash row
        sc = np.zeros((128, NCH), np.float32)
        for ei in range(E):
            n = int(cnt_ec[c, ei])
            l = sel[c][ei]
            w = selw[c][ei]
            jj = np.arange(n)
            gcol = off16[ei] + jj // 16
            grow = jj % 16
            gidx[grow, gcol] = l
            sidx[grow, gcol] = l
            sc[jj % 128, choff[ei] + jj // 128] = w
        in_maps.append({
            "x2": np.ascontiguousarray(x2_all[perm[c]]),
            "wa": wa,
            "gidx": np.tile(gidx, (8, 1)),
            "sidx": np.tile(sidx, (8, 1)),
            "sc": sc,
            "bias_b": bias_b,
        })
    return caps, bias_zero, in_maps, perm, tv, ti, counts


def get_nc(caps, bias_zero):
    key = (caps, bias_zero)
    if key not in _NC_CACHE:
        _NC_CACHE[key] = _build(caps, bias_zero)
    return _NC_CACHE[key]


def kernel(x, gate_weights, experts_weight, bias):
    from concourse.bass_utils import run_bass_kernel_spmd

    caps, bias_zero, in_maps, perm, tv, ti, counts = prepare(
        x, gate_weights, experts_weight, bias)
    nc = get_nc(caps, bias_zero)
    out = run_bass_kernel_spmd(nc, in_maps, core_ids=list(range(NCORES)))
    res_name = "acc" if bias_zero else "res"
    res = np.empty((B_T, D, 2), np.float32)
    for c in range(NCORES):
        res[perm[c]] = out.results[c][res_name][:TL].reshape(TL, D, 2)
    return res, ti, tv, counts[:, None, None]


# revision 37
# speedup vs baseline: 1.2320x; 1.2320x over previous
"""Expert-choice MoE with complex-valued experts + ModReLU, on 8 trn2 NeuronCores.

Strategy (token-sharded, no collectives):
  - Host computes the gating scores and jax.lax.top_k exactly as the reference
    does (CPU backend: reproducible, and it doubles as the routing / sharding
    decision, yielding the topk_idx / topk_scores / counts outputs directly).
  - Tokens are assigned to cores by a balancer so every (expert, core) pair
    gets exactly 128 of the expert's 1024 selected tokens -> zero GEMM padding.
  - The complex GEMM runs in split precision: x and W are each decomposed on
    the host into hi+lo bf16 parts (x = xh + xl to ~2^-18 relative); the
    device computes xh*wh + xh*wl + xl*wh with fp32 PSUM accumulation, which
    carries ~1e-5 relative error (fp32-envelope) at 3/4 the PE cost of native
    fp32 matmul, and makes x 2-byte so dma_gather(transpose=True) can gather
    AND transpose the selected tokens in one DMA (host pre-de-interleaves
    re/im so gathered partition chunks are the GEMM contraction chunks).
  - Per expert: gather-transpose xh/xl, 48 bf16 matmuls into yr/yi PSUM,
    gate-scale on the PSUM->SBUF ACT copy, then a CCE dma_scatter_add into a
    pre-zeroed per-core accumulator in HBM (expert-serialized: duplicate
    destination rows never race).
  - bias == 0 makes ModReLU the identity to within 1.5e-8 absolute
    (ratio = relu(m)/max(m,1e-8) = m/m = 1.0 exactly for m >= 1e-8, and
    |res| <= |out| < 1.5e-8 otherwise), so acc IS the result.  A general
    on-device ModReLU phase is kept for bias != 0.
  - Host scatters the 8 per-core result shards back through the balancer
    permutation.
"""

import numpy as np

B_T, D, E = 16384, 512, 16
NCORES = 8
TL = B_T // NCORES      # tokens per core
DRI = 2 * D             # re/im row length
KSEL = B_T // E         # top-k per expert
KPC = KSEL // NCORES    # per-(expert, core) target load = 128

_NC_CACHE = {}


def _round_up(v, m):
    return -(-v // m) * m


def _build(caps, bias_zero, repeat=1):
    """Build the per-core Bass program (same NEFF runs on all 8 cores).
    repeat>1 re-runs the whole accumulate pipeline (acc ends up repeat x the
    result) — used for steady-state wall-clock timing."""
    import concourse.bacc as bacc
    import concourse.mybir as mybir
    import concourse.tile as tile

    f32 = mybir.dt.float32
    bf16 = mybir.dt.bfloat16
    i16 = mybir.dt.int16
    AF = mybir.ActivationFunctionType

    ch = [-(-c // 128) for c in caps]                  # m-chunks per expert
    off16 = np.concatenate([[0], np.cumsum([c // 16 for c in caps])]).astype(int)
    choff = np.concatenate([[0], np.cumsum(ch)]).astype(int)
    GW16 = int(off16[-1])
    NCH = int(choff[-1])
    MAXCH = max(ch)
    CAPMAX = max(caps)

    nc = bacc.Bacc(None, target_bir_lowering=False)
    # x de-interleaved ([re0..511, im0..511] rows), hi/lo bf16 split packed
    # side by side: row = [xh(1024) | xl(1024)]
    x2_d = nc.dram_tensor("x2", [TL, 2 * DRI], bf16, kind="ExternalInput")
    # weights [E, 128, 16, 512] partition-major: 4 k-chunks each of
    # wrh | wrl | wih | wil, one contiguous 16KB DMA row per partition
    wa_d = nc.dram_tensor("wa", [E, 128, 16, D], bf16, kind="ExternalInput")
    gidx_d = nc.dram_tensor("gidx", [128, GW16], i16, kind="ExternalInput")
    sidx_d = nc.dram_tensor("sidx", [128, GW16], i16, kind="ExternalInput")
    sc_d = nc.dram_tensor("sc", [128, NCH], f32, kind="ExternalInput")
    bias_d = nc.dram_tensor("bias_b", [128, D], f32, kind="ExternalInput")
    # +16 trash rows: padded scatter instances land on row TL so their
    # (zero-valued) adds can never race a real token-0 add within a call
    acc_d = nc.dram_tensor("acc", [TL + 16, DRI], f32, kind="ExternalOutput")
    res_d = None
    if not bias_zero:
        res_d = nc.dram_tensor("res", [TL, DRI], f32, kind="ExternalOutput")

    with tile.TileContext(nc) as tc:
        with (
            tc.tile_pool(name="const", bufs=1) as constp,
            tc.tile_pool(name="w", bufs=2) as wp,
            tc.tile_pool(name="xt", bufs=2) as xtp,
            tc.tile_pool(name="y", bufs=3) as yp,
            tc.tile_pool(name="p2", bufs=2) as p2p,
            tc.tile_pool(name="yps", bufs=4, space="PSUM") as ypp,
        ):
            gidx_sb = constp.tile([128, GW16], i16)
            sidx_sb = constp.tile([128, GW16], i16)
            sc_sb = constp.tile([128, NCH], f32)
            nc.sync.dma_start(gidx_sb[:], gidx_d[:])
            nc.sync.dma_start(sidx_sb[:], sidx_d[:])
            nc.sync.dma_start(sc_sb[:], sc_d[:])
            bias_sb = None
            if not bias_zero:
                bias_sb = constp.tile([128, D], f32)
                nc.sync.dma_start(bias_sb[:], bias_d[:])

            for rep, e in [(r, e) for r in range(repeat) for e in range(E)]:
                cap = caps[e]
                che = ch[e]
                wa_t = wp.tile([128, 16, D], bf16, tag="wa", name=f"wa_{rep}_{e}")
                nc.sync.dma_start(wa_t[:], wa_d[e])
                w = {"wrh": wa_t[:, 0:4, :], "wrl": wa_t[:, 4:8, :],
                     "wih": wa_t[:, 8:12, :], "wil": wa_t[:, 12:16, :]}
                # negated wi parts (for yr -= xi*wi); sign-flip on DVE
                nwi_t = wp.tile([128, 8, D], bf16, tag="nwi", name=f"nwi_{rep}_{e}")
                nc.vector.tensor_scalar_mul(nwi_t[:], wa_t[:, 8:16, :], -1.0)
                w["nwih"] = nwi_t[:, 0:4, :]
                w["nwil"] = nwi_t[:, 4:8, :]

                # gather+transpose: out[p, c, i] = x2[row_i][c*128 + p]
                # c 0..3 -> xh re chunks, 4..7 -> xh im, 8..15 -> xl re/im
                x_t = xtp.tile([128, 16, cap], bf16, tag="xt", name=f"xt_{rep}_{e}")
                idx = gidx_sb[:, off16[e]:off16[e + 1]]
                nc.gpsimd.dma_gather(x_t[:], x2_d[:], idx,
                                     num_idxs=cap, num_idxs_reg=cap,
                                     elem_size=2 * DRI, transpose=True)
                xh_t = x_t[:, 0:8, :]
                xl_t = x_t[:, 8:16, :]

                y_sb = yp.tile([128, MAXCH, DRI], f32, tag="y")
                yv = y_sb[:].rearrange("p c (d two) -> p c two d", two=2)
                for j in range(che):
                    m = min(128, cap - 128 * j)
                    sl = slice(128 * j, 128 * j + m)
                    yr_ps = ypp.tile([128, D], f32, tag="y")
                    yi_ps = ypp.tile([128, D], f32, tag="y")
                    # yr = xr*wr - xi*wi, split into hi/lo cross terms
                    yr_terms = [("xh", 0, "wrh"), ("xh", 0, "wrl"),
                                ("xl", 0, "wrh"), ("xh", 4, "nwih"),
                                ("xh", 4, "nwil"), ("xl", 4, "nwih")]
                    yi_terms = [("xh", 0, "wih"), ("xh", 0, "wil"),
                                ("xl", 0, "wih"), ("xh", 4, "wrh"),
                                ("xh", 4, "wrl"), ("xl", 4, "wrh")]
                    for ps, terms in ((yr_ps, yr_terms), (yi_ps, yi_terms)):
                        n = len(terms) * 4
                        k = 0
                        for xs, coff, wn in terms:
                            xt = xh_t if xs == "xh" else xl_t
                            for kb in range(4):
                                nc.tensor.matmul(
                                    ps[:m, :], xt[:, coff + kb, sl],
                                    w[wn][:, kb, :],
                                    start=(k == 0), stop=(k == n - 1))
                                k += 1
                    col = int(choff[e]) + j
                    nc.scalar.activation(yv[:m, j, 0, :], yr_ps[:m, :], AF.Copy,
                                         scale=sc_sb[:m, col:col + 1])
                    nc.scalar.activation(yv[:m, j, 1, :], yi_ps[:m, :], AF.Copy,
                                         scale=sc_sb[:m, col:col + 1])
                mlast = cap - 128 * (che - 1)
                if mlast < 128:
                    nc.vector.memset(y_sb[:, che - 1, :], 0.0)
                # Padded instances (only when caps exceed the balanced 128)
                # carry zero y rows and scatter into the trash row TL.
                nc.gpsimd.dma_scatter_add(
                    acc_d[:],
                    y_sb[:, :che, :],
                    sidx_sb[:, off16[e]:off16[e + 1]],
                    num_idxs=cap,
                    num_idxs_reg=cap,
                    elem_size=DRI,
                )

            if not bias_zero:
                for j in range(TL // 128):
                    a_sb = p2p.tile([128, DRI], f32, tag="a")
                    nc.sync.dma_start(a_sb[:], acc_d[j * 128:(j + 1) * 128, :])
                    av = a_sb[:].rearrange("p (d two) -> p two d", two=2)
                    re, im = av[:, 0, :], av[:, 1, :]
                    m2r = p2p.tile([128, D], f32, tag="m2r")
                    m2i = p2p.tile([128, D], f32, tag="m2i")
                    nc.scalar.square(m2r[:], re)
                    nc.scalar.square(m2i[:], im)
                    m2 = p2p.tile([128, D], f32, tag="m2")
                    nc.vector.tensor_add(m2[:], m2r[:], m2i[:])
                    mag = p2p.tile([128, D], f32, tag="mag")
                    nc.scalar.sqrt(mag[:], m2[:])
                    ratio = p2p.tile([128, D], f32, tag="ratio")
                    t = p2p.tile([128, D], f32, tag="t")
                    nc.vector.tensor_add(t[:], mag[:], bias_sb[:])
                    nc.vector.tensor_scalar_max(t[:], t[:], 0.0)
                    sm = p2p.tile([128, D], f32, tag="sm")
                    nc.vector.tensor_scalar_max(sm[:], mag[:], 1e-8)
                    rcp = p2p.tile([128, D], f32, tag="rcp")
                    nc.vector.reciprocal(rcp[:], sm[:])
                    nc.vector.tensor_mul(ratio[:], t[:], rcp[:])
                    res_sb = p2p.tile([128, DRI], f32, tag="res")
                    rv = res_sb[:].rearrange("p (d two) -> p two d", two=2)
                    nc.vector.tensor_mul(rv[:, 0, :], re, ratio[:])
                    nc.vector.tensor_mul(rv[:, 1, :], im, ratio[:])
                    nc.sync.dma_start(res_d[j * 128:(j + 1) * 128, :], res_sb[:])

    nc.finalize()
    return nc


def _host_routing(x, gate_weights):
    """Mirror the reference gating for numpy-typed inputs: with np arrays the
    reference's `x.reshape(b_t, 2*d) @ gate_weights` is a NUMPY matmul, so
    compute it the same way (bit-identical scores).  top_k runs on the jax
    CPU backend — deterministic, spec tie-breaking (lower index first)."""
    import jax

    b_t, d, _ = x.shape
    e = gate_weights.shape[1]
    k = max(1, b_t // e)
    scores = x.reshape(b_t, 2 * d) @ gate_weights
    try:
        cpu = jax.devices("cpu")[0]
    except Exception:
        cpu = None
    ctx = jax.default_device(cpu) if cpu is not None else _nullcontext()
    with ctx:
        topk_scores, topk_idx = jax.lax.top_k(scores.T, k)
        return np.asarray(topk_scores), np.asarray(topk_idx)


class _nullcontext:
    def __enter__(self):
        return None

    def __exit__(self, *a):
        return False


def _balance(ti):
    """Assign tokens to cores (2048 each) so each expert's 1024 selected
    tokens split as close to 128-per-core as possible (eliminates GEMM
    padding). Returns perm[NCORES][TL] = global token ids per core slot."""
    members = [[] for _ in range(B_T)]
    for ei in range(E):
        for t in ti[ei]:
            members[int(t)].append(ei)
    active = [t for t in range(B_T) if members[t]]
    active.sort(key=lambda t: -len(members[t]))
    load = np.zeros((NCORES, E), np.int64)
    ntok = np.zeros(NCORES, np.int64)
    assign = np.full(B_T, -1, np.int64)
    for t in active:
        es = members[t]
        best, bkey = None, None
        for c in range(NCORES):
            if ntok[c] >= TL:
                continue
            if all(load[c][e] < KPC for e in es):
                key = (max(load[c][e] for e in es), ntok[c])
                if best is None or key < bkey:
                    best, bkey = c, key
        if best is None:
            # overflow fallback: least-overloaded core (caps grow past 128)
            best = min((c for c in range(NCORES) if ntok[c] < TL),
                       key=lambda c: sum(max(0, load[c][e] + 1 - KPC)
                                         for e in es))
        assign[t] = best
        ntok[best] += 1
        for e in es:
            load[best][e] += 1
    free = iter([t for t in range(B_T) if assign[t] < 0])
    perm = []
    for c in range(NCORES):
        mine = [t for t in range(B_T) if assign[t] == c]
        while len(mine) < TL:
            mine.append(next(free))
        perm.append(np.sort(np.array(mine, np.int64)))
    return perm


def _split_bf16(a):
    import ml_dtypes
    hi = a.astype(ml_dtypes.bfloat16)
    lo = (a - hi.astype(np.float32)).astype(ml_dtypes.bfloat16)
    return hi, lo


def prepare(x, gate_weights, experts_weight, bias):
    """Host-side routing + input packing. Returns (caps, bias_zero, in_maps,
    perm, tv, ti, counts)."""
    x = np.ascontiguousarray(np.asarray(x, dtype=np.float32))
    gate_weights = np.asarray(gate_weights, dtype=np.float32)
    experts_weight = np.asarray(experts_weight, dtype=np.float32)
    bias = np.asarray(bias, dtype=np.float32)

    tv, ti = _host_routing(x, gate_weights)
    counts = np.bincount(ti.reshape(-1), minlength=B_T).astype(np.float32)
    w_inst = (tv / np.maximum(counts[ti], 1.0)).astype(np.float32)

    perm = _balance(ti)
    tok_core = np.empty(B_T, np.int64)
    tok_slot = np.empty(B_T, np.int64)
    for c in range(NCORES):
        tok_core[perm[c]] = c
        tok_slot[perm[c]] = np.arange(TL)

    core = tok_core[ti]
    loc = tok_slot[ti]

    sel = [[None] * E for _ in range(NCORES)]
    selw = [[None] * E for _ in range(NCORES)]
    cnt_ec = np.zeros((NCORES, E), np.int64)
    for ei in range(E):
        for c in range(NCORES):
            msk = core[ei] == c
            l = loc[ei][msk]
            w = w_inst[ei][msk]
            o = np.argsort(l, kind="stable")
            sel[c][ei] = l[o].astype(np.int64)
            selw[c][ei] = w[o]
            cnt_ec[c, ei] = l.size

    # gather(transpose=True) requires num_idxs % 128 == 0
    caps = tuple(int(max(128, _round_up(int(cnt_ec[:, ei].max()), 128)))
                 for ei in range(E))
    ch = [-(-c // 128) for c in caps]
    off16 = np.concatenate([[0], np.cumsum([c // 16 for c in caps])]).astype(int)
    choff = np.concatenate([[0], np.cumsum(ch)]).astype(int)
    GW16 = int(off16[-1])
    NCH = int(choff[-1])

    # weights: [E, D, D] -> [E, 128, 4, D] partition-major, then hi/lo bf16,
    # packed as [E, 128, 16, D] = per-partition [wrh | wrl | wih | wil]
    wr = np.ascontiguousarray(
        experts_weight[:, :, :, 0].reshape(E, 4, 128, D).transpose(0, 2, 1, 3))
    wi = np.ascontiguousarray(
        experts_weight[:, :, :, 1].reshape(E, 4, 128, D).transpose(0, 2, 1, 3))
    wrh, wrl = _split_bf16(wr)
    wih, wil = _split_bf16(wi)
    wa = np.ascontiguousarray(
        np.concatenate([wrh, wrl, wih, wil], axis=2))

    # x: de-interleave re/im per row, hi/lo bf16 split packed side by side
    x_de = np.ascontiguousarray(
        x.reshape(B_T, D, 2).transpose(0, 2, 1).reshape(B_T, DRI))
    xh_all, xl_all = _split_bf16(x_de)
    x2_all = np.ascontiguousarray(np.concatenate([xh_all, xl_all], axis=1))

    bias_zero = bool(np.all(bias == 0.0))
    bias_b = np.broadcast_to(bias[None, :], (128, D)).copy()

    in_maps = []
    for c in range(NCORES):
        gidx = np.zeros((16, GW16), np.int16)
        sidx = np.full((16, GW16), TL, np.int16)  # pads -> trash row
        sc = np.zeros((128, NCH), np.float32)
        for ei in range(E):
            n = int(cnt_ec[c, ei])
            l = sel[c][ei]
            w = selw[c][ei]
            jj = np.arange(n)
            gcol = off16[ei] + jj // 16
            grow = jj % 16
            gidx[grow, gcol] = l
            sidx[grow, gcol] = l
            sc[jj % 128, choff[ei] + jj // 128] = w
        in_maps.append({
            "x2": np.ascontiguousarray(x2_all[perm[c]]),
            "wa": wa,
            "gidx": np.tile(gidx, (8, 1)),
            "sidx": np.tile(sidx, (8, 1)),
            "sc": sc,
            "bias_b": bias_b,
        })
    return caps, bias_zero, in_maps, perm, tv, ti, counts


def get_nc(caps, bias_zero):
    key = (caps, bias_zero)
    if key not in _NC_CACHE:
        _NC_CACHE[key] = _build(caps, bias_zero)
    return _NC_CACHE[key]


def kernel(x, gate_weights, experts_weight, bias):
    from concourse.bass_utils import run_bass_kernel_spmd

    caps, bias_zero, in_maps, perm, tv, ti, counts = prepare(
        x, gate_weights, experts_weight, bias)
    nc = get_nc(caps, bias_zero)
    out = run_bass_kernel_spmd(nc, in_maps, core_ids=list(range(NCORES)))
    res_name = "acc" if bias_zero else "res"
    res = np.empty((B_T, D, 2), np.float32)
    for c in range(NCORES):
        res[perm[c]] = out.results[c][res_name][:TL].reshape(TL, D, 2)
    return res, ti, tv, counts[:, None, None]
